# revision 17
# baseline (speedup 1.0000x reference)
"""Trainium2 Bass kernel for nn_MCQuantiles (ThreeCompNode SNN scan).

Strategy (8 NeuronCores, data-parallel over batch):
- Each core takes 8 batches x 32 samples = 256 rows of the B*S axis.
- Everything runs in "transposed space": feature dims on SBUF partitions,
  batch-rows on the free dim. All transposes/swizzles are done host-side for
  free; every DMA is a flat contiguous [128, X] block.
- The input matmuls (te @ Wa.T, se @ Wb.T) don't depend on the recurrence, so
  apical is computed for pairs of time steps with N=512 moving operands.
- Membrane recurrences use 2^t-scaled state so each update is a single fused
  scalar_tensor_tensor op reading the matmul result straight from PSUM:
      alpha_t = alpha_{t-1} + 2^t * apical_t         (alpha = 2^{t+1} ma)
      mu_t    = mu_{t-1} + 0.5*alpha_t + 0.5*beta_t  (mu = 2^{t+1} ms)
      spike   <=> mu > 2^{t+1}
- Layer-1 spikes are fed to the W1 matmul as q = NOT(spike) with the
  rowsum(W1)+b1 constant folded in host-side (h = c1 - q @ W1.T).
- Layer-2 spikes sp2 are fed directly to the W2 matmul; out accumulates in a
  persistent PSUM bank over all T, evicted once with scale 1/T + bias b2.
- Matmuls run in bf16 (full PE rate). Binary spike inputs are bf16-exact; the
  LIF threshold margin (|ml|max ~0.35 vs th 0.5) makes output spikes immune to
  bf16 rounding of the weights.
"""
import numpy as np
import ml_dtypes

import bass_rust
import concourse.bass as bass
import concourse.mybir as mybir
from concourse.bass_utils import run_bass_kernel_spmd
from concourse.tile import TileContext

# ----- problem constants (hardcoded per contract) -----
T, B, S = 8, 64, 32
DS = DT = 3136
F = H = 512
L = 18
N_CORES = 8
NB = B // N_CORES              # 8 batches per core
R = NB * S                     # 256 rows per core
KD = 3200                      # 3136 padded to 25 k-tiles of 128
NK = KD // 128                 # 25
NPAIR = T // 2                 # 4 step pairs
NG = F // 128                  # 4 f-tiles (= h-tiles)

# column offsets inside the bf16 weight walls [128, *]
WA_COLS = NK * F               # wallA: apical weights only
O_WB = 0                       # wallB: basal weights, NK*F cols
O_SE = O_WB + NK * F           # state embeddings, NK*T*NB cols
O_W1 = O_SE + NK * T * NB      # W1.T, NG*H cols
O_W2 = O_W1 + NG * H           # W2.T, NG*L cols
WB_COLS = O_W2 + NG * L

F32 = mybir.dt.float32
BF16 = mybir.dt.bfloat16
OP = mybir.AluOpType


def _patch_tile_drain():
    """This walrus build allows a single sync-wait per TPB_CTRL Drain; Tile's
    kernel-tail drain attaches one wait per active logical proc. Split them
    across a chain of drains."""
    def _patched(self, tick_clock, wait_clock):
        nc = self.nc
        drain_inst = nc.sync.drain()
        wait_clock.add_sem_waits(
            drain_inst.ins, bass_rust.ScopedClock({None: tick_clock.global_clock})
        )
        si = drain_inst.ins.sync_info
        if si is not None and len(si.on_wait) > 1:
            waits = list(si.on_wait)
            drain_inst.ins.sync_info = mybir.SyncInfo(
                on_wait=waits[:1], on_update=list(si.on_update)
            )
            for w in waits[1:]:
                extra = nc.sync.drain()
                extra.ins.sync_info = mybir.SyncInfo(on_wait=[w], on_update=[])
        nc.all_engine_barrier()
        popped = nc._tile_sem_poison_stack.pop()
        assert popped is self._sem_poison
        nc.clear_and_free_semaphores(list(self.sems.allocated().values()))
        nc.all_engine_barrier()

    TileContext._drain_and_barrier = _patched


def _split_excess_waits(nc, limit=1):
    """Walrus here rejects instructions carrying more than ~1 sync-wait. Move
    excess waits onto same-engine NoOps inserted just before the instruction."""
    for fn in nc.m.functions:
        for bb in fn.blocks:
            new = []
            changed = False
            for inst in bb.instructions:
                si = getattr(inst, "sync_info", None)
                ow = list(si.on_wait) if si is not None and si.on_wait else []
                if len(ow) > limit:
                    extra = ow[limit:]
                    for j in range(0, len(extra), limit):
                        nop = mybir.InstNoOp(
                            name=f"{inst.name}-ws{j}", ins=[], outs=[]
                        )
                        nop.engine = inst.engine
                        nop.sync_info = mybir.SyncInfo(
                            on_wait=extra[j : j + limit], on_update=[]
                        )
                        new.append(nop)
                    inst.sync_info = mybir.SyncInfo(
                        on_wait=ow[:limit], on_update=list(si.on_update)
                    )
                    changed = True
                new.append(inst)
            if changed:
                bb.set_instructions(new) if hasattr(bb, "set_instructions") else None
                if not hasattr(bb, "set_instructions"):
                    try:
                        bb.instructions[:] = new
                    except TypeError:
                        bb.instructions = new


def build_nc(with_b1=False):  # with_b1 kept for API compat; unused
    _patch_tile_drain()
    nc = bass.Bass()

    teT = nc.declare_dram_parameter("teT", [NPAIR, 128, NK * 2 * R], BF16, isOutput=False)
    wallA = nc.declare_dram_parameter("wallA", [128, WA_COLS], BF16, isOutput=False)
    wallB = nc.declare_dram_parameter("wallB", [128, WB_COLS], BF16, isOutput=False)
    cons = nc.declare_dram_parameter("cons", [128, NG * T + 1 + 2 * T], F32, isOutput=False)
    out = nc.declare_dram_parameter("out", [L, R], F32, isOutput=True)

    with TileContext(nc) as tc:
        with (
            tc.tile_pool(name="wpool", bufs=1) as wpool,
            tc.tile_pool(name="tepool", bufs=2) as tepool,
            tc.tile_pool(name="state", bufs=1) as state,
            tc.tile_pool(name="qpool", bufs=2) as qpool,
            tc.tile_pool(name="appool", bufs=4, space="PSUM") as appool,
            tc.tile_pool(name="hpool", bufs=3, space="PSUM") as hpool,
            tc.tile_pool(name="opool", bufs=1, space="PSUM") as opool,
        ):
            # ---- resident weights/constants ----
            NCH = 5  # k-tiles per DMA chunk
            NCHUNK = NK // NCH
            wallA_c = []
            for c in range(NCHUNK):
                wa_ck = wpool.tile(
                    [128, NCH * F], BF16, tag=f"wallA{c}", name=f"wa_ck{c}"
                )
                wallA_c.append(wa_ck)
                nc.sync.dma_start(
                    wa_ck[:], wallA[:, c * NCH * F : (c + 1) * NCH * F]
                )
            # wallB/cons DMAs are emitted inside the pair-0 body (after the
            # apical matmuls) so they don't steal HBM bandwidth at startup.
            wallB_sb = wpool.tile([128, WB_COLS], BF16, tag="wallB", name="wallB_sb")
            cons_sb = wpool.tile([128, NG * T + 1 + 2 * T], F32, tag="cons", name="cons_sb")

            def waT(k, g):
                c, kk = divmod(k, NCH)
                return wallA_c[c][:, kk * F + g * 128 : kk * F + (g + 1) * 128]

            def wbT(k, g):
                return wallB_sb[:, O_WB + k * F + g * 128 : O_WB + k * F + (g + 1) * 128]

            def seT(k):
                return wallB_sb[:, O_SE + k * T * NB : O_SE + (k + 1) * T * NB]

            def w1T(k, g):
                return wallB_sb[:, O_W1 + k * H + g * 128 : O_W1 + k * H + (g + 1) * 128]

            def w2T(k):
                return wallB_sb[:, O_W2 + k * L : O_W2 + (k + 1) * L]

            def c1s_ap(g, t):
                return cons_sb[:, g * T + t : g * T + t + 1]

            b2_ap = cons_sb[0:L, NG * T : NG * T + 1]

            def th1_ap(t):  # -(2^{t+1})
                c = NG * T + 1 + t
                return cons_sb[:, c : c + 1]

            def th2_ap(t):  # -(2^t)
                c = NG * T + 1 + T + t
                return cons_sb[:, c : c + 1]

            # ---- state tiles ----
            A = [[state.tile([128, R], F32, tag=f"A{g}_{p}", name=f"A{g}_{p}")
                  for p in range(2)] for g in range(NG)]
            M = [state.tile([128, R], F32, tag=f"M{g}", name=f"M{g}") for g in range(NG)]
            ML = [state.tile([128, R], F32, tag=f"ML{g}", name=f"ML{g}") for g in range(NG)]
            Bsc = [state.tile([128, T * NB], F32, tag=f"Bsc{g}", name=f"Bsc{g}")
                   for g in range(NG)]

            o_psum = opool.tile([L, R], F32, tag="o", name="o_psum")

            # ---- main time loop over step pairs ----
            basal_emitted = False
            for pair in range(NPAIR):
                te_c = []
                for c in range(NCHUNK):
                    tck = tepool.tile(
                        [128, NCH * 2 * R], BF16, tag=f"te{c}", name=f"te_ck{c}"
                    )
                    te_c.append(tck)
                    nc.sync.dma_start(
                        tck[:],
                        teT[pair][:, c * NCH * 2 * R : (c + 1) * NCH * 2 * R],
                    )

                ap_psum = [
                    appool.tile([128, 2 * R], F32, tag="ap", name="ap_psum")
                    for _ in range(NG)
                ]
                for c in range(NCHUNK):
                    for g in range(NG):
                        for kk in range(NCH):
                            k = c * NCH + kk
                            nc.tensor.matmul(
                                ap_psum[g][:],
                                lhsT=waT(k, g),
                                rhs=te_c[c][:, kk * 2 * R : (kk + 1) * 2 * R],
                                start=(k == 0),
                                stop=(k == NK - 1),
                            )

                if not basal_emitted:
                    nc.sync.dma_start(wallB_sb[:], wallB[:])
                    nc.sync.dma_start(cons_sb[:], cons[:])
                    # basal: bs[f, t*NB+b] for all t, then beta prefix scan.
                    # Emitted after pair-0 apical so PE starts on wallA+te0 and
                    # doesn't stall on the wallB DMA.
                    basal_emitted = True
                    for g in range(NG):
                        bs_psum = hpool.tile([128, T * NB], F32, tag="hq", name="bs_psum")
                        for k in range(NK):
                            nc.tensor.matmul(
                                bs_psum[:],
                                lhsT=wbT(k, g),
                                rhs=seT(k),
                                start=(k == 0),
                                stop=(k == NK - 1),
                            )
                        # beta_t = beta_{t-1} + 2^t * basal_t  (beta = 2^{t+1} mb)
                        for t in range(T):
                            dst = Bsc[g][:, t * NB : (t + 1) * NB]
                            srcp = bs_psum[:, t * NB : (t + 1) * NB]
                            if t == 0:
                                nc.vector.tensor_scalar(dst, srcp, 0.5, None, OP.mult)
                            else:
                                nc.vector.scalar_tensor_tensor(
                                    dst, srcp, float(2 ** (t - 1)),
                                    Bsc[g][:, (t - 1) * NB : t * NB],
                                    OP.mult, OP.add,
                                )

                # Early alpha updates for BOTH sub-steps: consumes the PSUM
                # slices immediately so the banks free up for the next pair.
                for sub in range(2):
                    t = 2 * pair + sub
                    for g in range(NG):
                        apq = ap_psum[g][:, sub * R : (sub + 1) * R]
                        if t == 0:
                            nc.vector.tensor_scalar(
                                A[g][0][:], apq, 0.5, None, OP.mult
                            )
                        else:
                            nc.vector.scalar_tensor_tensor(
                                A[g][t % 2][:], apq, float(2 ** (t - 1)),
                                A[g][1 - t % 2][:], OP.mult, OP.add,
                            )

                for sub in range(2):
                    t = 2 * pair + sub
                    sc_t = float(2 ** t)
                    q_b16 = []
                    for g in range(NG):
                        At = A[g][t % 2]
                        # mu += alpha_half ; mu += beta_half_bc
                        if t == 0:
                            nc.vector.tensor_copy(M[g][:], At[:])
                        else:
                            nc.vector.tensor_tensor(M[g][:], At[:], M[g][:], OP.add)
                        b_bc = (
                            Bsc[g][:, t * NB : (t + 1) * NB]
                            .unsqueeze(2)
                            .broadcast_to([128, NB, S])
                        )
                        m_v = M[g].rearrange("p (b s) -> p b s", s=S)
                        nc.vector.tensor_tensor(m_v, b_bc, m_v, OP.add)
                        # q = NOT spike = (mu <= th) in {0,1} bf16; reset by mult
                        qg = qpool.tile([128, R], BF16, tag=f"q{g}", name="qg")
                        q_b16.append(qg)
                        nc.vector.tensor_scalar(
                            qg[:], M[g][:], float(2 ** (t + 1)), None, OP.is_le
                        )
                        nc.vector.tensor_tensor(M[g][:], M[g][:], qg[:], OP.mult)

                    hq_psum = []
                    for g in range(NG):
                        ps = hpool.tile([128, R], F32, tag="hq", name="hq_psum")
                        hq_psum.append(ps)
                        for k in range(NG):
                            nc.tensor.matmul(
                                ps[:],
                                lhsT=w1T(k, g),
                                rhs=q_b16[k][:],
                                start=(k == 0),
                                stop=(k == NG - 1),
                            )

                    sp2_b16 = []
                    for g in range(NG):
                        # lambda += 2^t*(c1 - hq); hq = q @ W1.T, c1 = rowsum(W1)+b1
                        if t == 0:
                            nc.vector.tensor_scalar(
                                ML[g][:], hq_psum[g][:], -1.0, None, OP.mult
                            )
                        else:
                            nc.vector.scalar_tensor_tensor(
                                ML[g][:], hq_psum[g][:], -sc_t, ML[g][:], OP.mult, OP.add
                            )
                        nc.scalar.activation(
                            ML[g][:], ML[g][:],
                            mybir.ActivationFunctionType.Identity,
                            bias=c1s_ap(g, t), scale=1.0,
                        )
                        # sp2 = (lambda > th) in {0,1} bf16
                        spg = qpool.tile([128, R], BF16, tag=f"sp2{g}", name="spg")
                        sp2_b16.append(spg)
                        nc.vector.tensor_scalar(spg[:], ML[g][:], sc_t, None, OP.is_gt)
                        # reset: lambda = (lambda <= th) * lambda, in place
                        nc.vector.scalar_tensor_tensor(
                            ML[g][:], ML[g][:], sc_t, ML[g][:], OP.is_le, OP.mult
                        )

                    for k in range(NG):
                        nc.tensor.matmul(
                            o_psum[:],
                            lhsT=w2T(k),
                            rhs=sp2_b16[k][:],
                            start=(t == 0 and k == 0),
                            stop=(t == T - 1 and k == NG - 1),
                        )

            # ---- final eviction: out = o_psum / T + b2 ----
            out_sb = state.tile([L, R], F32, tag="out_sb", name="out_sb")
            nc.scalar.activation(
                out_sb[:], o_psum[:],
                mybir.ActivationFunctionType.Identity,
                bias=b2_ap, scale=1.0 / T,
            )
            nc.sync.dma_start(out[:], out_sb[:])

    return nc


def _swizzle_kmaj(a, cols):
    """[KD-like rows, cols] fp -> [128, nk*cols] bf16 with [p, k*cols+c]=a[k*128+p, c]"""
    bf = ml_dtypes.bfloat16
    nk = a.shape[0] // 128
    return np.ascontiguousarray(
        a.reshape(nk, 128, cols).transpose(1, 0, 2).reshape(128, nk * cols).astype(bf)
    )


def prep_in_maps(inputs):
    """Host-side shard + transpose + pad + cast. Returns list of per-core dicts."""
    se = np.asarray(inputs["state_embedding"], np.float32)
    te = np.asarray(inputs["tau_embedding"], np.float32)
    Wb = np.asarray(inputs["Wb"], np.float32)
    Wa = np.asarray(inputs["Wa"], np.float32)
    W1 = np.asarray(inputs["W1"], np.float32)
    b1 = np.asarray(inputs["b1"], np.float32)
    W2 = np.asarray(inputs["W2"], np.float32)
    b2 = np.asarray(inputs["b2"], np.float32)
    bf = ml_dtypes.bfloat16

    def padk(a):  # pad feature axis 0 from 3136 to KD
        o = np.zeros((KD,) + a.shape[1:], a.dtype)
        o[: a.shape[0]] = a
        return o

    wallA = _swizzle_kmaj(padk(Wa.T), F)
    wallB = np.empty((128, WB_COLS), bf)
    wallB[:, O_WB : O_WB + NK * F] = _swizzle_kmaj(padk(Wb.T), F)
    wallB[:, O_W1 : O_W1 + NG * H] = _swizzle_kmaj(np.ascontiguousarray(W1.T), H)
    wallB[:, O_W2 : O_W2 + NG * L] = _swizzle_kmaj(np.ascontiguousarray(W2.T), L)

    cons = np.zeros((128, NG * T + 1 + 2 * T), np.float32)
    c1 = W1.sum(axis=1) + b1
    for g in range(NG):
        for t in range(T):
            cons[:, g * T + t] = c1[g * 128 : (g + 1) * 128] * (2.0 ** t)
    cons[:L, NG * T] = b2
    for t in range(T):
        cons[:, NG * T + 1 + t] = -(2.0 ** (t + 1))
        cons[:, NG * T + 1 + T + t] = -(2.0 ** t)

    in_maps = []
    for i in range(N_CORES):
        # teT: [NPAIR, 128, NK*2R] with [pair, p, k*512 + (sub*R+r)] = te[t, row, d]
        tei = te[:, i * R : (i + 1) * R, :]       # [T, R, DT]
        tei = tei.reshape(NPAIR, 2 * R, DT)       # [pair, sub*R+r, d]
        tei_p = np.zeros((NPAIR, 2 * R, KD), np.float32)
        tei_p[:, :, :DT] = tei
        teT = np.ascontiguousarray(
            tei_p.reshape(NPAIR, 2 * R, NK, 128)
            .transpose(0, 3, 2, 1)                # [pair, p, k, n]
            .reshape(NPAIR, 128, NK * 2 * R)
            .astype(bf)
        )
        # seT region of wall: [p, k*T*NB + t*NB+b] = se[t, batch, d]
        sei = se[:, i * NB : (i + 1) * NB, :]     # [T, NB, DS]
        seT = padk(np.ascontiguousarray(sei.reshape(T * NB, DS).T))  # [KD, T*NB]
        wallB_i = wallB.copy()
        wallB_i[:, O_SE : O_SE + NK * T * NB] = _swizzle_kmaj(seT, T * NB)
        in_maps.append(dict(teT=teT, wallA=wallA, wallB=wallB_i, cons=cons))
    return in_maps


def assemble_out(core_outs):
    """[N_CORES][L, R] -> [B, L, S]"""
    full = np.stack([np.asarray(o, np.float32) for o in core_outs], axis=0)
    full = full.reshape(N_CORES, L, NB, S).transpose(0, 2, 1, 3)
    return np.ascontiguousarray(full.reshape(B, L, S))


_NC_CACHE = {}


def get_nc(with_b1=False):
    key = "nc"
    if key not in _NC_CACHE:
        last = None
        for _ in range(6):
            try:
                _NC_CACHE[key] = build_nc(with_b1=with_b1)
                break
            except Exception as e:  # rare scheduler-order race-detector trip
                last = e
        else:
            raise last
    return _NC_CACHE[key]


def run_sharded(in_maps, with_b1=False, trace=False, **kw):
    nc = get_nc(with_b1=with_b1)
    if not getattr(nc, "_waits_split", False):
        _split_excess_waits(nc)
        nc._waits_split = True
    res = run_bass_kernel_spmd(
        nc, in_maps, core_ids=list(range(N_CORES)), trace=trace, **kw
    )
    return res


def kernel(**inputs):
    in_maps = prep_in_maps(inputs)
    with_b1 = bool(np.any(np.asarray(inputs["b1"], np.float32)))
    res = run_sharded(in_maps, with_b1=with_b1)
    return assemble_out([res.results[i]["out"] for i in range(N_CORES)])
